# revision 2
# baseline (speedup 1.0000x reference)
"""Trainium2 Bass kernel for batched multi-head self-attention.

Reference computation (per batch element b):
    qkv = x @ w_qkv.T                  # [N, 3C]
    q, k, v = split/reshape to heads   # H=16 heads, d=64
    attn = softmax(q @ k.T / sqrt(d))
    out = (attn @ v) reshaped back     # [N, C]
    y = out @ w_proj.T + b_proj

Sharding: pure data-parallel over batch B=8 across the 8 NeuronCores
(one batch element per core, weights replicated, no collectives).

On-device layout (everything transposed so matmuls contract over the
partition axis with no on-device transposes of the inputs):
  - xT      [C, N]   (host pre-transposed, bf16)
  - wqkvT   [C, 3C]  (host pre-transposed, bf16)
  - wprojT  [C, C]   (host pre-transposed, bf16)
  - x^T and the v-columns of w_qkv are fused host-side into one "xw"
    tensor: one DMA per contraction tile.

Performance structure (TimelineSim 209.4us, down from the 229.2us
all-bf16 baseline; PE busy ~191.6us = 459776 moving rows x 0.4167ns):
  - The PE cost model charges moving-rows per instruction (F), so a
    stage's cost is output_elements x K_tiles / M_width: full 128-wide
    output partitions everywhere is what matters.  The classic AV
    orientation (v stationary [keys,65]) wastes half the PE (M=65).
  - AV TRANSPOSED: pt is the STATIONARY operand [keys, n-tile 128], v
    the moving [keys, 65] (64 dims + ones column).  Output av[n, d+1]
    uses M=128: AV drops from 131072 to 66560 rows (-27us of PE).
  - The ones column lands the softmax sums at av[:, 64] - a
    per-partition column - so normalization is a DVE reciprocal +
    broadcast-multiply (no cross-partition broadcast, no gpsimd, no
    SBUF->SBUF DMAs on the critical path).
  - av accumulators (4 n-tiles x (d+1) per bank, one bank per head) are
    memset-zeroed and accumulated with start=False: a start=True matmul
    zeroes its whole PSUM bank, wiping sibling accumulators that share
    it (verified empirically - sub-bank accumulation groups MUST avoid
    start=True).
  - Both heads' QK^T scores go to one 2-bank PSUM tile [128, 2x512];
    ONE wide exp per m-step ([128,1024] f32->bf16, 1038ns) instead of
    two 612ns halves.  This keeps ACT (134us total) under the PE and
    is required: unmerged exp makes attention ACT-bound beyond what
    projection work can fill (+35us end-to-end).
  - Attention output [n, d] returns to the projection's [d, n] layout
    via the XBAR DMA transpose (dma_start_transpose, [128,128] blocks,
    112ns each on the DMA queues - no PE cost), writing directly into
    ao[pair].
  - PSUM budget: st 2x2 banks + av 2x1 + acc 2x1 = 8.  PSUM slot
    rotation follows tile allocation order, so same-tag users are
    effectively scheduled by emission order; the v-projection borrows
    the (pre-attention idle) st/av slots for 6-way chain concurrency
    against the DMA arrival rate of the fused xw tiles.
  - Emission interleaves attention with the next pair's q/k projection
    so the scheduler fills exp-bound PE bubbles; output projection is
    emitted last and streams through the acc slots during/after the
    last pair.
  - Dummy warm-up matmuls on memset data cover the initial input-DMA
    wait and complete the PE p-state ramp before real work arrives.
    Startup is input-bandwidth-bound (phase A's 4MB working set at
    360B/ns), not PE-bound.
  - Output stored bf16 (halves store transfers); host converts to f32.
    Total rel err 6.0e-3 vs the 2e-2 gate.

Rejected experimentally (TimelineSim regressions): fp8 anywhere
(3-6e-2 rel err, fails the gate), routing transposes or y-stores
through the ACT DGE queue (+0.5..31us - blocks exp dispatch), y-bias
on ACT/Pool, normalize-by-divide on Pool, two-stage projection with
banked partials, splitting the first xw DMA, unmerged exp.
"""

import os
import sys

for _p in ("/opt/trn_rl_repo", "/root/.axon_site/_ro/trn_rl_repo"):
    if os.path.isdir(_p) and _p not in sys.path:
        sys.path.insert(0, _p)
        break

import numpy as np
import ml_dtypes

import concourse.bass as bass
import concourse.bacc as bacc
import concourse.tile as tile
import concourse.mybir as mybir
from concourse import bass_utils

BF16 = mybir.dt.bfloat16
F32 = mybir.dt.float32
AF = mybir.ActivationFunctionType

B, N, C, H = 8, 1024, 1024, 16
D = C // H            # 64 head dim
P = 128               # partitions
CT = C // P           # 8 contraction tiles
NT2 = N // 512        # 2 n-tiles of 512
MT = N // P           # 8 m-tiles of 128
PAIRS = H // 2        # 8 head pairs
SCALE = float(D) ** -0.5
N_CORES = 8

_cache = {}


def _build():
    nc = bacc.Bacc("TRN2", target_bir_lowering=False, debug=False,
                   enable_asserts=False, num_devices=N_CORES)

    xw_d = nc.dram_tensor("xw", [C, 2 * N], BF16, kind="ExternalInput")
    wqkvT_d = nc.dram_tensor("wqkvT", [C, 3 * C], BF16, kind="ExternalInput")
    wprojT_d = nc.dram_tensor("wprojT", [C, C], BF16, kind="ExternalInput")
    bias_d = nc.dram_tensor("bias", [P, CT], F32, kind="ExternalInput")
    outT_d = nc.dram_tensor("outT", [C, N], BF16, kind="ExternalOutput")

    with tile.TileContext(nc) as tc:
        with (
            tc.tile_pool(name="res", bufs=1) as rp,
            tc.tile_pool(name="work", bufs=2) as wp,
            tc.tile_pool(name="ps", bufs=1, space="PSUM") as pp,
        ):
            def chain_ps(name, tag, bufs):
                """Chain accumulator [P,512] view; st slots are 2 banks."""
                if tag == "st":
                    t = pp.tile([P, 2, 512], F32, name=name, tag="st", bufs=2)
                    return t[:, 0, :]
                t = pp.tile([P, 512], F32, name=name, tag=tag, bufs=bufs)
                return t[:]

            # ---------------- PE warm-up ----------------
            warm_a = wp.tile([P, 512], BF16, name="warm_a", tag="warm_a",
                             bufs=1)
            nc.gpsimd.memset(warm_a[:], 0.25)
            warm_ps = pp.tile([P, 512], F32, name="warm_ps", tag="acc",
                              bufs=2)
            for _ in range(6):
                nc.tensor.matmul(warm_ps[:], warm_a[:, 0:P], warm_a[:],
                                 start=True, stop=True)

            # ---------------- resident inputs ----------------
            xT = []
            wqv = []
            for i in range(CT):
                t = rp.tile([P, 2 * N], BF16, name=f"xw{i}", tag=f"xw{i}")
                nc.sync.dma_start(t[:], xw_d.ap()[i * P:(i + 1) * P, :])
                xT.append(t[:, 0:N])
                wqv.append(t[:, N:2 * N])
            wqk = []
            for i in range(CT):
                t = rp.tile([P, 2 * C], BF16, name=f"wqk{i}", tag=f"wqk{i}")
                nc.sync.dma_start(t[:],
                                  wqkvT_d.ap()[i * P:(i + 1) * P, 0:2 * C])
                wqk.append(t)
            wpj = []
            for i in range(CT):
                t = rp.tile([P, C], BF16, name=f"wpj{i}", tag=f"wpj{i}")
                nc.sync.dma_start(t[:], wprojT_d.ap()[i * P:(i + 1) * P, :])
                wpj.append(t)
            bias_t = rp.tile([P, CT], F32, name="bias_t", tag="bias")
            nc.sync.dma_start(bias_t[:], bias_d.ap())

            # ---------------- result tiles ----------------
            qT = [rp.tile([P, N], BF16, name=f"qT{i}", tag=f"qT{i}")
                  for i in range(PAIRS)]
            kT = [rp.tile([P, N], BF16, name=f"kT{i}", tag=f"kT{i}")
                  for i in range(PAIRS)]
            vt = [[rp.tile([P, 8, D + 1], BF16, name=f"v{m}_{j}",
                           tag=f"v{m}_{j}") for j in range(2)]
                  for m in range(MT)]
            ao = [rp.tile([P, N], BF16, name=f"ao{i}", tag=f"ao{i}")
                  for i in range(PAIRS)]

            for m in range(MT):
                for j in range(2):
                    nc.vector.memset(vt[m][j][:, :, D:D + 1], 1.0)

            # ---------------- phase A(v): v projection ----------------
            # 6-way chain concurrency: the attention tags (st/ava/avb)
            # are idle before the pair loop, and phase A is gated by the
            # xw DMA arrival rate (1456ns/tile), needing >= 6 chains at
            # 213ns/step to keep the PE from starving.
            for m in range(MT):
                for j in range(2):
                    vtag, vbufs = (("acc", 2), ("st", 2), ("ava", 1),
                                   ("acc", 2), ("st", 2),
                                   ("avb", 1))[(2 * m + j) % 6]
                    ps = chain_ps(f"accv{m}_{j}", vtag, vbufs)
                    for c in range(CT):
                        nc.tensor.matmul(
                            ps,
                            xT[c][:, m * P:(m + 1) * P],
                            wqv[c][:, j * 512:(j + 1) * 512],
                            start=(c == 0), stop=(c == CT - 1),
                        )
                    nc.vector.tensor_copy(
                        vt[m][j][:, :, 0:D],
                        ps.rearrange("p (h d) -> p h d", d=D),
                    )

            # ------- interleaved: q/k projection + attention per pair -------
            for pr in range(PAIRS):
                for which, dst in ((0, qT[pr]), (1, kT[pr])):
                    o0 = which * C + pr * P
                    for n2 in range(NT2):
                        nsl = slice(n2 * 512, (n2 + 1) * 512)
                        ps = chain_ps(f"accqk{pr}_{which}_{n2}", "acc", 2)
                        for c in range(CT):
                            nc.tensor.matmul(
                                ps,
                                wqk[c][:, o0:o0 + P],
                                xT[c][:, nsl],
                                start=(c == 0), stop=(c == CT - 1),
                            )
                        nc.vector.tensor_copy(dst[:, nsl], ps)

                # attention for this pair, one 512-wide n2 half at a time
                for n2 in range(NT2):
                    nsl = slice(n2 * 512, (n2 + 1) * 512)
                    # 2 accumulator banks: 4 n-tiles x (d+1) per head
                    av = [pp.tile([P, 4, D + 1], F32, name=f"av{pr}_{n2}_{h}",
                                  tag=("ava", "avb")[h], bufs=1)
                          for h in range(2)]
                    for h in range(2):
                        nc.vector.memset(av[h][:], 0.0)
                    for m in range(MT):
                        msl = slice(m * P, (m + 1) * P)
                        pt2 = wp.tile([P, 2, 512], BF16,
                                      name=f"pt{pr}_{m}_{n2}", tag="pt",
                                      bufs=6)
                        st2 = pp.tile([P, 2, 512], F32,
                                      name=f"st{pr}_{m}_{n2}", tag="st",
                                      bufs=2)
                        for h in range(2):
                            psl = slice(h * 64, (h + 1) * 64)
                            nc.tensor.matmul(
                                st2[:, h, :],
                                kT[pr][psl, msl],
                                qT[pr][psl, nsl],
                                start=True, stop=True,
                                tile_position=(h * 64, 0),
                            )
                        nc.scalar.activation(pt2[:], st2[:], AF.Exp,
                                             scale=SCALE)
                        for h in range(2):
                            head = 2 * pr + h
                            vtile = vt[m][head // 8]
                            for ntl in range(4):
                                nc.tensor.matmul(
                                    av[h][:, ntl, :],
                                    pt2[:, h, ntl * P:(ntl + 1) * P],
                                    vtile[:, head % 8, :],
                                    start=False, stop=(m == MT - 1),
                                    skip_group_check=True,
                                )
                    # normalize on DVE: per-partition recip + broadcast mul
                    ao_nd = wp.tile([P, 4, 2, D], BF16,
                                    name=f"aond{pr}_{n2}", tag="aond",
                                    bufs=4)
                    rec = wp.tile([P, 2, 4, 1], F32, name=f"rec{pr}_{n2}",
                                  tag="rec", bufs=4)
                    for h in range(2):
                        nc.vector.reciprocal(rec[:, h],
                                             av[h][:, :, D:D + 1])
                        nc.vector.tensor_mul(
                            ao_nd[:, :, h, :],
                            av[h][:, :, 0:D],
                            rec[:, h].broadcast_to([P, 4, D]),
                        )
                    for ntl in range(4):
                        gnt = n2 * 4 + ntl
                        nc.sync.dma_start_transpose(
                            ao[pr][:, gnt * P:(gnt + 1) * P],
                            ao_nd[:, ntl].rearrange("p h d -> p (h d)"),
                        )

            # ---------------- phase C: output projection ----------------
            for n2 in range(NT2):
                for ot in range(CT):
                    nsl = slice(n2 * 512, (n2 + 1) * 512)
                    ps = chain_ps(f"accy{ot}_{n2}", "acc", 2)
                    for pr in range(PAIRS):
                        nc.tensor.matmul(
                            ps,
                            wpj[pr][:, ot * P:(ot + 1) * P],
                            ao[pr][:, nsl],
                            start=(pr == 0), stop=(pr == PAIRS - 1),
                        )
                    yt = wp.tile([P, 512], BF16, name=f"y{ot}_{n2}", tag="y",
                                 bufs=3)
                    nc.vector.tensor_scalar_add(yt[:], ps,
                                                bias_t[:, ot:ot + 1])
                    nc.sync.dma_start(outT_d.ap()[ot * P:(ot + 1) * P, nsl],
                                      yt[:])

    nc.compile()
    return nc


def get_nc():
    if "nc" not in _cache:
        _cache["nc"] = _build()
    return _cache["nc"]


def kernel(x, w_qkv, w_proj, b_proj):
    x = np.asarray(x, dtype=np.float32)
    w_qkv = np.asarray(w_qkv, dtype=np.float32)
    w_proj = np.asarray(w_proj, dtype=np.float32)
    b_proj = np.asarray(b_proj, dtype=np.float32)

    bf = ml_dtypes.bfloat16
    wqkvT = np.ascontiguousarray(w_qkv.T).astype(bf)     # [C, 3C]
    wprojT = np.ascontiguousarray(w_proj.T).astype(bf)   # [C, C]
    bias = np.ascontiguousarray(b_proj.reshape(CT, P).T).astype(np.float32)

    in_maps = []
    wqv_host = wqkvT[:, 2 * C:]                          # [C, C] v columns
    for b in range(N_CORES):
        xT = np.ascontiguousarray(x[b].T).astype(bf)     # [C, N]
        xw = np.ascontiguousarray(np.concatenate([xT, wqv_host], axis=1))
        in_maps.append({"xw": xw, "wqkvT": wqkvT, "wprojT": wprojT,
                        "bias": bias})

    nc = get_nc()
    _cache["in_maps"] = in_maps
    res = bass_utils.run_bass_kernel_spmd(nc, in_maps,
                                          core_ids=list(range(N_CORES)))
    out = np.empty((B, N, C), dtype=np.float32)
    for b in range(N_CORES):
        out[b] = res.results[b]["outT"].T.astype(np.float32)
    return out
